# revision 5
# baseline (speedup 1.0000x reference)
"""Trainium2 Bass kernel for nn_BatchConv1d (dynamic grouped conv attention).

Reference computation (per batch b):
    kernel = (q @ W_kernel + b_kernel).reshape(Lq, C, KW)      # dynamic conv kernels
    bias   = (q @ W_bias + b_bias)[:, 0]
    kpad   = zero-pad k along L by PAD=1
    a[i,j] = sum_{c,w} kernel[i,c,w] * kpad[j+w,c] + bias[i] + bias_b

Strategy: data-parallel over B=8 (one batch per NeuronCore). Per core:
  Stage 1 (PE): kernelT_ext[cw, i] = sum_d Wp_ext[d, cw] * qT[d, i]
     with host-permuted Wp_ext so cw = w*C + c, plus a 13th M-tile holding
     W_bias (row 0) -> bias row. qT built on-chip via PE transposes.
  Stage 2 (PE): out[i, j] = sum_{ct,w} kernelT[w*4+ct][:, i] . kT_pad[ct][:, j+w]
     kT_pad is the on-chip transpose of k with one zero column on each side,
     so the 3 window shifts are just free-dim offsets. b_kernel is folded into
     kernelT during the PSUM->SBUF copy; bias row added via a 13th accumulation
     matmul against an all-ones tile.
All matmuls run as float32r (TF32-like, full PE rate).
"""

import numpy as np
from contextlib import ExitStack

import concourse.bass as bass
import concourse.mybir as mybir
import concourse.tile as tile
from concourse import bacc
from concourse.bass_utils import run_bass_kernel_spmd
from concourse.masks import make_identity

F32 = mybir.dt.float32
F32R = mybir.dt.float32r

B, Lq, Lk, D, C, KW = 8, 1024, 1024, 512, 512, 3
CW = C * KW            # 1536
CW_EXT = CW + 128      # 1664 (13th tile: W_bias col + 127 zero cols)
NT_I = Lq // 128       # 8 i-tiles
NT_D = D // 128        # 4 d-tiles
NT_CW = CW_EXT // 128  # 13 cw-tiles (12 kernel + 1 bias)
NJ = 2                 # j chunks of 512

_CACHE = {}


def _build():
    nc = bacc.Bacc(target_bir_lowering=False, debug=False)

    q_in = nc.dram_tensor("q_in", [Lq, D], F32, kind="ExternalInput").ap()
    k_in = nc.dram_tensor("k_in", [Lk, C], F32, kind="ExternalInput").ap()
    wp_in = nc.dram_tensor("wp_in", [D, CW_EXT], F32, kind="ExternalInput").ap()
    bkp_in = nc.dram_tensor("bkp_in", [128, NT_CW - 1], F32, kind="ExternalInput").ap()
    bconst_in = nc.dram_tensor("bconst_in", [1, 1], F32, kind="ExternalInput").ap()
    out = nc.dram_tensor("out", [Lq, Lk], F32, kind="ExternalOutput").ap()

    with tile.TileContext(nc) as tc:
        with ExitStack() as ctx:
            persist = ctx.enter_context(tc.tile_pool(name="persist", bufs=1))
            qk_pool = ctx.enter_context(tc.tile_pool(name="qk", bufs=1))
            out_pool = ctx.enter_context(tc.tile_pool(name="outp", bufs=4))

            ident = persist.tile([128, 128], F32, tag="ident")
            make_identity(nc, ident[:])

            # ---- small constants -------------------------------------------------
            bkp_col = persist.tile([128, NT_CW - 1], F32, tag="bkp")
            nc.sync.dma_start(bkp_col[:], bkp_in[:])
            bconst_col = persist.tile([128, 1], F32, tag="bconst")
            nc.vector.memset(bconst_col[:], 0.0)
            nc.sync.dma_start(bconst_col[0:1, :], bconst_in[:])

            # memset can't target f32r tiles; stage via fp32 and copy-cast
            ones_f32 = persist.tile([128, 1024], F32, tag="ones_f32")
            nc.vector.memset(ones_f32[:], 1.0)
            ones_t = persist.tile([128, 1024], F32R, tag="ones")
            nc.vector.tensor_copy(ones_t[:], ones_f32[:])
            zero_col = persist.tile([128, 1], F32, tag="zero_col")
            nc.vector.memset(zero_col[:], 0.0)

            # ---- load q, k (natural layout) -------------------------------------
            q_tiled = q_in.rearrange("(t p) d -> t p d", p=128)
            k_tiled = k_in.rearrange("(t p) c -> t p c", p=128)
            q_sb = [qk_pool.tile([128, D], F32, tag=f"q{t}", name=f"q{t}") for t in range(NT_I)]
            k_sb = [qk_pool.tile([128, C], F32, tag=f"k{t}", name=f"k{t}") for t in range(NT_I)]
            for t in range(NT_I):
                nc.sync.dma_start(q_sb[t][:], q_tiled[t])
            for t in range(NT_I):
                nc.sync.dma_start(k_sb[t][:], k_tiled[t])

            # ---- load + round Wp to f32r ----------------------------------------
            wp_tiled = wp_in.rearrange("(t p) m -> t p m", p=128)
            wp_sb = []
            for t in range(NT_D):
                w_raw = qk_pool.tile([128, CW_EXT], F32, tag=f"wraw{t}")
                nc.sync.dma_start(w_raw[:], wp_tiled[t])
                w_r = persist.tile([128, CW_EXT], F32R, tag=f"wp{t}")
                nc.vector.tensor_copy(w_r[:], w_raw[:])
                wp_sb.append(w_r)

            # ---- transpose q -> qT (f32r), k -> kT_pad (f32r) -------------------
            with tc.tile_pool(name="tpsum", bufs=4, space="PSUM") as tpsum:
                qT = [persist.tile([128, Lq], F32R, tag=f"qT{d}", name=f"qT{d}") for d in range(NT_D)]
                for it in range(NT_I):
                    for dt in range(NT_D):
                        pt = tpsum.tile([128, 128], F32, tag="tp")
                        nc.tensor.transpose(
                            pt[:], q_sb[it][:, dt * 128:(dt + 1) * 128], ident[:]
                        )
                        nc.vector.tensor_copy(
                            qT[dt][:, it * 128:(it + 1) * 128], pt[:]
                        )

                kT_pad = [
                    persist.tile([128, Lk + 2], F32R, tag=f"kT{c}", name=f"kT{c}")
                    for c in range(NT_D)
                ]
                for ct in range(NT_D):
                    nc.vector.tensor_copy(kT_pad[ct][:, 0:1], zero_col[:])
                    nc.vector.tensor_copy(kT_pad[ct][:, Lk + 1:Lk + 2], zero_col[:])
                for jt in range(NT_I):
                    for ct in range(NT_D):
                        pt = tpsum.tile([128, 128], F32, tag="tp")
                        nc.tensor.transpose(
                            pt[:], k_sb[jt][:, ct * 128:(ct + 1) * 128], ident[:]
                        )
                        nc.vector.tensor_copy(
                            kT_pad[ct][:, 1 + jt * 128:1 + (jt + 1) * 128], pt[:]
                        )

            # ---- stage 1: kernelT_ext[cw, i] ------------------------------------
            kernelT = [
                persist.tile([128, Lq], F32R, tag=f"kern{t}", name=f"kern{t}") for t in range(NT_CW)
            ]
            with tc.tile_pool(name="s1psum", bufs=4, space="PSUM") as s1psum:
                for mt in range(NT_CW):
                    for njc in range(NJ):
                        ps = s1psum.tile([128, 512], F32, tag="s1")
                        for dt in range(NT_D):
                            nc.tensor.matmul(
                                ps[:],
                                wp_sb[dt][:, mt * 128:(mt + 1) * 128],
                                qT[dt][:, njc * 512:(njc + 1) * 512],
                                start=(dt == 0),
                                stop=(dt == NT_D - 1),
                            )
                        dst = kernelT[mt][:, njc * 512:(njc + 1) * 512]
                        if mt < NT_CW - 1:
                            nc.vector.tensor_scalar_add(
                                dst, ps[:], bkp_col[:, mt:mt + 1]
                            )
                        else:
                            # bias tile: row 0 = q @ W_bias (+ b_bias + bias_b)
                            nc.vector.tensor_scalar_add(
                                dst, ps[:], bconst_col[:]
                            )

            # ---- stage 2: out[i, j] ---------------------------------------------
            with tc.tile_pool(name="s2psum", bufs=4, space="PSUM") as s2psum:
                for it in range(NT_I):
                    for jc in range(NJ):
                        ps = s2psum.tile([128, 512], F32, tag="s2")
                        first = True
                        for w in range(KW):
                            for ct in range(NT_D):
                                nc.tensor.matmul(
                                    ps[:],
                                    kernelT[w * NT_D + ct][:, it * 128:(it + 1) * 128],
                                    kT_pad[ct][:, jc * 512 + w:jc * 512 + w + 512],
                                    start=first,
                                    stop=False,
                                )
                                first = False
                        # bias row via all-ones moving operand
                        nc.tensor.matmul(
                            ps[:],
                            kernelT[NT_CW - 1][:, it * 128:(it + 1) * 128],
                            ones_t[:, jc * 512:(jc + 1) * 512],
                            start=False,
                            stop=True,
                        )
                        o_sb = out_pool.tile([128, 512], F32, tag="osb")
                        nc.vector.tensor_copy(o_sb[:], ps[:])
                        nc.sync.dma_start(
                            out[it * 128:(it + 1) * 128, jc * 512:(jc + 1) * 512],
                            o_sb[:],
                        )

    nc.compile()
    return nc


def _get_nc():
    if "nc" not in _CACHE:
        _CACHE["nc"] = _build()
    return _CACHE["nc"]


def _prepare_in_maps(q, k, W_kernel, b_kernel, W_bias, b_bias, bias_b):
    q = np.asarray(q, dtype=np.float32)
    k = np.asarray(k, dtype=np.float32)
    W_kernel = np.asarray(W_kernel, dtype=np.float32)
    b_kernel = np.asarray(b_kernel, dtype=np.float32)
    W_bias = np.asarray(W_bias, dtype=np.float32)
    b_bias = np.asarray(b_bias, dtype=np.float32)
    bias_b = np.asarray(bias_b, dtype=np.float32)

    # host-side permutation: Wp[:, w*C + c] = W_kernel[:, c*KW + w]
    Wp = W_kernel.reshape(D, C, KW).transpose(0, 2, 1).reshape(D, CW)
    Wp_ext = np.concatenate(
        [Wp, W_bias.reshape(D, 1), np.zeros((D, 127), np.float32)], axis=1
    )
    Wp_ext = np.ascontiguousarray(Wp_ext, dtype=np.float32)
    bkp = b_kernel.reshape(C, KW).T.reshape(CW)
    bkp_col = np.ascontiguousarray(bkp.reshape(NT_CW - 1, 128).T, dtype=np.float32)
    bconst = np.array([[b_bias.reshape(-1)[0] + bias_b.reshape(-1)[0]]], np.float32)

    return [
        {
            "q_in": np.ascontiguousarray(q[b]),
            "k_in": np.ascontiguousarray(k[b]),
            "wp_in": Wp_ext,
            "bkp_in": bkp_col,
            "bconst_in": bconst,
        }
        for b in range(B)
    ]


def kernel(q, k, W_kernel, b_kernel, W_bias, b_bias, bias_b):
    in_maps = _prepare_in_maps(q, k, W_kernel, b_kernel, W_bias, b_bias, bias_b)
    res = run_bass_kernel_spmd(_get_nc(), in_maps, core_ids=list(range(B)))
    return np.stack([res.results[b]["out"] for b in range(B)], axis=0)


def kernel_profiled(q, k, W_kernel, b_kernel, W_bias, b_bias, bias_b, **kw):
    """Like kernel() but with NTFF tracing; returns (output, BassKernelResults)."""
    in_maps = _prepare_in_maps(q, k, W_kernel, b_kernel, W_bias, b_bias, bias_b)
    res = run_bass_kernel_spmd(
        _get_nc(), in_maps, core_ids=list(range(B)), trace=True, **kw
    )
    out = np.stack([res.results[b]["out"] for b in range(B)], axis=0)
    return out, res
